# revision 13
# baseline (speedup 1.0000x reference)
"""Causal self-attention (B=2, T=2048, C=1024, H=16, D=64) on 8 trn2 cores.

Sharding: tensor-parallel on heads - 2 heads per core. Each core computes
QKV projection for its 2 heads, causal softmax attention, and its heads'
slice of the output projection (a rank-128 partial sum of the full output).
The host pre-transposes x to [B, C, T], slices the weights per core, and
sums the 8 partial outputs (+ proj bias) at the end. Partials are stored
bf16 to halve output DMA.

v2 layout (per core):
  - q-subtiles of 512 columns; S^T for BOTH heads per k-chunk computed by a
    row-tiled concurrent matmul pair (heads' K=64 contractions in array row
    groups 0-1 / 2-3) into one [128, 2, 512] fp32 PSUM tile (2 banks).
  - ONE exp activation per chunk covers both heads via a strided 3D AP;
    causal triangle zeroed by one DVE multiply against a doubled tri mask.
  - O^T accumulates per head as (V|1)-chunk.T @ P^T; ones column makes row
    64 the softmax denominator. O MMs lag one chunk behind exp so the PE
    streams S(kc+1)/O(kc-1)/filler while ACT computes exp(kc).
  - V transposed to token-major via DMA XBAR transpose (no PE/DVE cost).
  - Normalization: fast reciprocal straight off the PSUM denominator row,
    GpSimd partition-broadcast, DVE multiply into un[128, 512] (2 heads).
  - Projection: K=128 matmuls per 128-token strip, PSUM -> bf16 SBUF ->
    DRAM. QKV for later batches/halves and projection of finished subtiles
    run as PE filler inside the ACT-bound attention loop.
"""

from collections import deque

import numpy as np

import concourse.bass as bass
import concourse.tile as tile
from concourse import bacc, mybir
from concourse.bass_utils import run_bass_kernel_spmd

dt = mybir.dt
AF = mybir.ActivationFunctionType

B, T, C, H, D = 2, 2048, 1024, 16, 64
NCORES = 8
HPC = H // NCORES          # heads per core = 2
QS = 512                   # q-subtile (columns of S^T/O^T psum tiles)
KC = 128                   # k chunk (partition dim of S^T)
SCALE = 1.0 / 8.0          # 1/sqrt(D)

_CACHE = {}


def _emit(tc):
    from contextlib import ExitStack
    with ExitStack() as ctx:
        _emit_body(tc, ctx)


def _emit_body(tc, ctx):
    nc = tc.nc
    f32, bf16 = dt.float32, dt.bfloat16

    xT = nc.dram_tensor("xT", [B, C, T], bf16, kind="ExternalInput").ap()
    wqkv = nc.dram_tensor("wqkv", [C, 384], bf16, kind="ExternalInput").ap()
    bqkv = nc.dram_tensor("bqkv", [128, 3], f32, kind="ExternalInput").ap()
    wp = nc.dram_tensor("wp", [128, C], bf16, kind="ExternalInput").ap()
    tri2 = nc.dram_tensor("tri2", [128, 256], bf16, kind="ExternalInput").ap()
    outp = nc.dram_tensor("outp", [B, T, C], bf16, kind="ExternalOutput").ap()

    consts = ctx.enter_context(tc.tile_pool(name="consts", bufs=1))
    xpool = ctx.enter_context(tc.tile_pool(name="xpool", bufs=2))
    qkvpool = ctx.enter_context(tc.tile_pool(name="qkvpool", bufs=6))
    vtmpool = ctx.enter_context(tc.tile_pool(name="vtmpool", bufs=2))
    vscr = ctx.enter_context(tc.tile_pool(name="vscr", bufs=2))
    ptpool = ctx.enter_context(tc.tile_pool(name="ptpool", bufs=3))
    unormp = ctx.enter_context(tc.tile_pool(name="unormp", bufs=8))
    rows = ctx.enter_context(tc.tile_pool(name="rows", bufs=2))
    outsb = ctx.enter_context(tc.tile_pool(name="outsb", bufs=6))
    stp = ctx.enter_context(tc.tile_pool(name="stp", bufs=2, space="PSUM"))
    otp = ctx.enter_context(tc.tile_pool(name="otp", bufs=1, space="PSUM"))
    miscp = ctx.enter_context(tc.tile_pool(name="miscp", bufs=2, space="PSUM"))

    # constants / weights resident in SBUF; first-needed data DMA'd first:
    # (w, x) interleaved per kc so QKV(b0, tg0) starts after ~2 chunks land.
    w_sb = consts.tile([128, 8, 384], bf16, tag="w")
    wsrc = wqkv.rearrange("(k p) f -> p k f", p=128)
    b_sb = consts.tile([128, 3], f32, tag="b")
    wp_sb = consts.tile([128, C], bf16, tag="wp")
    tri_sb = consts.tile([128, 256], bf16, tag="tri")

    xps = [xpool.tile([128, 8, T], bf16, tag="xp", name=f"xp{b}")
           for b in range(B)]
    xsrcs = [xT[b].rearrange("(j p) t -> p j t", p=128) for b in range(B)]
    for kc in range(8):
        nc.sync.dma_start(out=w_sb[:, kc, :], in_=wsrc[:, kc, :])
        nc.sync.dma_start(out=xps[0][:, kc, 0:1024],
                          in_=xsrcs[0][:, kc, 0:1024])
    nc.sync.dma_start(out=b_sb, in_=bqkv)
    nc.sync.dma_start(out=tri_sb, in_=tri2)
    for b, tg in [(0, 1), (1, 0), (1, 1)]:
        t0 = tg * 1024
        for kc in range(8):
            nc.sync.dma_start(out=xps[b][:, kc, t0:t0 + 1024],
                              in_=xsrcs[b][:, kc, t0:t0 + 1024])
    nc.sync.dma_start(out=wp_sb, in_=wp)

    filler = deque()

    def pop_filler(n=1):
        for _ in range(n):
            if filler:
                filler.popleft()[1]()

    def drain_until(key):
        """Pop everything up to and including the last item tagged `key`."""
        if not any(k == key for k, _ in filler):
            return
        while filler:
            k, th = filler.popleft()
            th()
            if k == key and not any(k2 == key for k2, _ in filler):
                break

    qkv_t = {}

    def make_qkv(b):
        """qkvT tiles + thunks per (tg, m, n): one 8-MM dense chain each."""
        dsts = [qkvpool.tile([128, T], bf16, tag="qkv", name=f"qkv{b}_{m}")
                for m in range(3)]
        qkv_t[b] = dsts
        thunks = {0: [], 1: []}
        for tg in range(2):
            for m in (2, 1, 0):    # v first: unblocks the vt transpose
                for n in range(2):
                    def th(m=m, tg=tg, n=n):
                        t0 = tg * 1024 + n * QS
                        pg = miscp.tile([128, QS], f32, tag="mm", name="pg")
                        for kc in range(8):
                            nc.tensor.matmul(
                                pg[:, :],
                                w_sb[:, kc, 128 * m:128 * m + 128],
                                xps[b][:, kc, t0:t0 + QS],
                                start=(kc == 0), stop=(kc == 7),
                            )
                        nc.vector.tensor_scalar_add(
                            dsts[m][:, t0:t0 + QS], pg[:, :],
                            b_sb[:, m:m + 1])
                    thunks[tg].append(th)
        return thunks

    vt_t = {}

    def make_vt(b):
        vt = vtmpool.tile([128, 16, HPC, 65], bf16, tag="vtm", name=f"vt{b}")
        vt_t[b] = vt
        nc.vector.memset(vt[:, :, :, 64:65], 1.0)

        def make_th(g, h):
            def th():
                # XBAR transpose needs a contiguous output tile; DVE then
                # scatters into the interleaved (v|1) layout.
                sc = vscr.tile([128, 8, 64], bf16, tag="vscr")
                nc.sync.dma_start_transpose(
                    sc[:, :, :],
                    qkv_t[b][2][64 * h:64 * h + 64, 1024 * g:1024 * g + 1024])
                nc.vector.tensor_copy(
                    out=vt[:, 8 * g:8 * g + 8, h, 0:64], in_=sc[:, :, :])
            return th
        return {g: [make_th(g, h) for h in range(HPC)] for g in range(2)}

    def make_proj(b, s, un):
        """Projection thunks, one per 128-token strip."""
        thunks = []
        for ts in range(QS // 128):
            def th(ts=ts):
                a0 = s * QS + ts * 128
                for ct in range(2):
                    pp = miscp.tile([128, 512], f32, tag="mm", name="pp")
                    nc.tensor.matmul(
                        pp[:, :],
                        un[:, ts * 128:(ts + 1) * 128],
                        wp_sb[:, ct * 512:(ct + 1) * 512],
                        start=True, stop=True,
                    )
                    ob = outsb.tile([128, 512], bf16, tag="osb")
                    if (ts * 2 + ct) % 4 == 3:  # 1/4 on ScalarE: balance DVE
                        nc.scalar.copy(ob[:, :], pp[:, :])
                    else:
                        nc.vector.tensor_copy(ob[:, :], pp[:, :])
                    nc.sync.dma_start(
                        out=outp[b, a0:a0 + 128, ct * 512:(ct + 1) * 512],
                        in_=ob[:, :])
            thunks.append(th)
        return thunks

    def emit_attention(b, s):
        qT_t, kT_t, vT_t = qkv_t[b]
        vt = vt_t[b]
        q0 = s * QS
        nkc = 4 * (s + 1)
        ot = [otp.tile([65, QS], f32, tag=f"ot{h}", name=f"ot{h}")
              for h in range(HPC)]

        def emit_o(kc):
            # One MM per (chunk, head) covering [ls:QS): the tri-masked pt
            # zeroes invalid diagonal-strip contributions, so no region
            # split is needed. kc=0 always covers the full [0:QS) width.
            ls = max(0, kc * KC - q0)
            last = kc == nkc - 1
            pt = pts[kc]
            for h in range(HPC):
                nc.tensor.matmul(
                    ot[h][:, ls:QS], vt[:, kc, h, :], pt[:, h, ls:QS],
                    start=(kc == 0), stop=last)

        pts = {}
        for kc in range(nkc):
            k0 = kc * KC
            ls = max(0, k0 - q0)
            st = stp.tile([128, HPC, QS], f32, tag="st")
            for h in range(HPC):
                nc.tensor.matmul(
                    st[:, h, ls:QS],
                    kT_t[64 * h:64 * h + 64, k0:k0 + KC],
                    qT_t[64 * h:64 * h + 64, q0 + ls:q0 + QS],
                    start=True, stop=True, tile_position=(64 * h, 0))
            pt = ptpool.tile([128, HPC, QS], bf16, tag="pt")
            pts[kc] = pt
            nc.scalar.activation(
                pt[:, :, ls:QS], st[:, :, ls:QS], AF.Exp, scale=SCALE)
            if kc >= 4 * s:  # diagonal chunk: zero invalid triangle, 2 heads
                nc.gpsimd.tensor_mul(
                    pt[:, :, ls:ls + 128], pt[:, :, ls:ls + 128],
                    tri_sb[:, :])
            # O lags one chunk so exp(kc) overlaps PE work
            if kc > 0:
                emit_o(kc - 1)
                pts.pop(kc - 1)
            pop_filler()
        emit_o(nkc - 1)
        pop_filler()

        # normalization: un[64h:64h+64] = ot_h[0:64] / ot_h[64]
        un = unormp.tile([128, QS], bf16, tag="un", name=f"un{b}{s}")
        se = rows.tile([1, 2 * QS], f32, tag="se", name="se")
        for h in range(HPC):
            nc.vector.tensor_copy(se[:, h * QS:(h + 1) * QS], ot[h][64:65, :])
        rc = rows.tile([1, 2 * QS], f32, tag="rc", name="rc")
        nc.vector.reciprocal_approx_fast(rc[:, :], se[:, :])
        for h in range(HPC):
            rb = rows.tile([64, QS], f32, tag=f"rb{h}", name=f"rb{h}")
            nc.gpsimd.partition_broadcast(rb[:, :], rc[:, h * QS:(h + 1) * QS])
            nc.vector.tensor_mul(
                un[64 * h:64 * h + 64, :], ot[h][0:64, :], rb[:, :])
        return un

    # front: batch 0, first half QKV + vt, emitted densely
    th0 = make_qkv(0)
    vth0 = make_vt(0)
    for th in th0[0]:
        th()
    for th in vth0[0]:
        th()

    # filler: remaining QKV/vt work, phased one step ahead of consumption
    th1 = make_qkv(1)
    vth1 = make_vt(1)
    filler.extend((("qkv", 1, 0), th) for th in th1[0])
    filler.extend((("vt", 1, 0), th) for th in vth1[0])
    filler.extend((("qkv", 0, 1), th) for th in th0[1])
    filler.extend((("vt", 0, 1), th) for th in vth0[1])
    filler.extend((("qkv", 1, 1), th) for th in th1[1])
    filler.extend((("vt", 1, 1), th) for th in vth1[1])

    held = []
    for b, s in [(0, 0), (0, 1), (1, 0), (1, 1),
                 (0, 2), (0, 3), (1, 2), (1, 3)]:
        g = 0 if s < 2 else 1
        drain_until(("qkv", b, g))
        drain_until(("vt", b, g))
        un = emit_attention(b, s)
        pthunks = make_proj(b, s, un)
        if (b, s) == (1, 3):
            held.extend(pthunks)
        else:
            filler.extend((("proj", b, s), th) for th in pthunks)

    while filler:
        pop_filler()
    for th in held:
        th()


def build():
    if "nc" in _CACHE:
        return _CACHE["nc"]
    nc = bacc.Bacc("TRN2", target_bir_lowering=False, debug=False,
                   num_devices=NCORES)
    with tile.TileContext(nc) as tc:
        _emit(tc)
    nc.compile()
    _CACHE["nc"] = nc
    return nc


def make_in_maps(x, qkv_w, qkv_b, proj_w):
    import ml_dtypes
    bf16 = ml_dtypes.bfloat16
    x = np.asarray(x, dtype=np.float32)
    qkv_w = np.asarray(qkv_w, dtype=np.float32)
    qkv_b = np.asarray(qkv_b, dtype=np.float32)
    proj_w = np.asarray(proj_w, dtype=np.float32)

    xT = np.ascontiguousarray(x.transpose(0, 2, 1)).astype(bf16)
    tri = (np.arange(128)[None, :] >= np.arange(128)[:, None]).astype(bf16)
    tri2 = np.ascontiguousarray(np.tile(tri, (1, 2)))

    in_maps = []
    for c in range(NCORES):
        s = 64 * HPC * c  # first feature row of this core's heads
        wq = qkv_w[:, s:s + 128]
        wk = qkv_w[:, C + s:C + s + 128]
        wv = qkv_w[:, 2 * C + s:2 * C + s + 128]
        wqkv_c = np.ascontiguousarray(
            np.concatenate([wq, wk, wv], axis=1)).astype(bf16)
        bqkv_c = np.ascontiguousarray(np.stack(
            [qkv_b[s:s + 128], qkv_b[C + s:C + s + 128],
             qkv_b[2 * C + s:2 * C + s + 128]], axis=1))
        wp_c = np.ascontiguousarray(proj_w[s:s + 128, :]).astype(bf16)
        in_maps.append({
            "xT": xT, "wqkv": wqkv_c, "bqkv": bqkv_c, "wp": wp_c,
            "tri2": tri2,
        })
    return in_maps


def kernel(x, qkv_w, qkv_b, proj_w, proj_b, _trace=False):
    nc = build()
    in_maps = make_in_maps(x, qkv_w, qkv_b, proj_w)
    res = run_bass_kernel_spmd(nc, in_maps, core_ids=list(range(NCORES)),
                               trace=_trace)
    acc = np.zeros((B, T, C), dtype=np.float64)
    for c in range(NCORES):
        acc += res.results[c]["outp"].astype(np.float64)
    acc += np.asarray(proj_b, dtype=np.float64)
    out = acc.astype(np.float32)
    _CACHE["last_results"] = res
    return out


# revision 14
# speedup vs baseline: 1.5073x; 1.5073x over previous
"""Causal self-attention (B=2, T=2048, C=1024, H=16, D=64) on 8 trn2 cores.

Sharding: tensor-parallel on heads - 2 heads per core. Each core computes
QKV projection for its 2 heads, causal softmax attention, and its heads'
slice of the output projection (a rank-128 partial sum of the full output).
The host pre-transposes x to [B, C, T], slices the weights per core, and
sums the 8 partial outputs (+ proj bias) at the end. Partials are stored
bf16 to halve output DMA.

v2 layout (per core):
  - q-subtiles of 512 columns; S^T for BOTH heads per k-chunk computed by a
    row-tiled concurrent matmul pair (heads' K=64 contractions in array row
    groups 0-1 / 2-3) into one [128, 2, 512] fp32 PSUM tile (2 banks).
  - ONE exp activation per chunk covers both heads via a strided 3D AP;
    causal triangle zeroed by one DVE multiply against a doubled tri mask.
  - O^T accumulates per head as (V|1)-chunk.T @ P^T; ones column makes row
    64 the softmax denominator. O MMs lag one chunk behind exp so the PE
    streams S(kc+1)/O(kc-1)/filler while ACT computes exp(kc).
  - V transposed to token-major via DMA XBAR transpose (no PE/DVE cost).
  - Normalization: fast reciprocal straight off the PSUM denominator row,
    GpSimd partition-broadcast, DVE multiply into un[128, 512] (2 heads).
  - Projection: K=128 matmuls per 128-token strip, PSUM -> bf16 SBUF ->
    DRAM. QKV for later batches/halves and projection of finished subtiles
    run as PE filler inside the ACT-bound attention loop.
"""

from collections import deque

import numpy as np

import concourse.bass as bass
import concourse.tile as tile
from concourse import bacc, mybir
from concourse.bass_utils import run_bass_kernel_spmd

dt = mybir.dt
AF = mybir.ActivationFunctionType

B, T, C, H, D = 2, 2048, 1024, 16, 64
NCORES = 8
HPC = H // NCORES          # heads per core = 2
QS = 512                   # q-subtile (columns of S^T/O^T psum tiles)
KC = 128                   # k chunk (partition dim of S^T)
SCALE = 1.0 / 8.0          # 1/sqrt(D)

_CACHE = {}


def _emit(tc):
    from contextlib import ExitStack
    with ExitStack() as ctx:
        _emit_body(tc, ctx)


def _emit_body(tc, ctx):
    nc = tc.nc
    f32, bf16 = dt.float32, dt.bfloat16

    xT = nc.dram_tensor("xT", [B, C, T], bf16, kind="ExternalInput").ap()
    wqkv = nc.dram_tensor("wqkv", [C, 384], bf16, kind="ExternalInput").ap()
    bqkv = nc.dram_tensor("bqkv", [128, 3], f32, kind="ExternalInput").ap()
    wp = nc.dram_tensor("wp", [128, C], bf16, kind="ExternalInput").ap()
    tri2 = nc.dram_tensor("tri2", [128, 256], bf16, kind="ExternalInput").ap()
    outp = nc.dram_tensor("outp", [B, T, C], bf16, kind="ExternalOutput").ap()

    consts = ctx.enter_context(tc.tile_pool(name="consts", bufs=1))
    xpool = ctx.enter_context(tc.tile_pool(name="xpool", bufs=2))
    qkvpool = ctx.enter_context(tc.tile_pool(name="qkvpool", bufs=6))
    vtmpool = ctx.enter_context(tc.tile_pool(name="vtmpool", bufs=2))
    vscr = ctx.enter_context(tc.tile_pool(name="vscr", bufs=2))
    ptpool = ctx.enter_context(tc.tile_pool(name="ptpool", bufs=3))
    unormp = ctx.enter_context(tc.tile_pool(name="unormp", bufs=8))
    rows = ctx.enter_context(tc.tile_pool(name="rows", bufs=2))
    outsb = ctx.enter_context(tc.tile_pool(name="outsb", bufs=6))
    stp = ctx.enter_context(tc.tile_pool(name="stp", bufs=2, space="PSUM"))
    otp = ctx.enter_context(tc.tile_pool(name="otp", bufs=1, space="PSUM"))
    miscp = ctx.enter_context(tc.tile_pool(name="miscp", bufs=2, space="PSUM"))

    # constants / weights resident in SBUF; first-needed data DMA'd first:
    # (w, x) interleaved per kc so QKV(b0, tg0) starts after ~2 chunks land.
    w_sb = consts.tile([128, 8, 384], bf16, tag="w")
    wsrc = wqkv.rearrange("(k p) f -> p k f", p=128)
    b_sb = consts.tile([128, 3], f32, tag="b")
    wp_sb = consts.tile([128, C], bf16, tag="wp")
    tri_sb = consts.tile([128, 256], bf16, tag="tri")

    xps = [xpool.tile([128, 8, T], bf16, tag="xp", name=f"xp{b}")
           for b in range(B)]
    xsrcs = [xT[b].rearrange("(j p) t -> p j t", p=128) for b in range(B)]
    for kc in range(8):
        nc.sync.dma_start(out=w_sb[:, kc, :], in_=wsrc[:, kc, :])
        nc.sync.dma_start(out=xps[0][:, kc, 0:1024],
                          in_=xsrcs[0][:, kc, 0:1024])
    nc.sync.dma_start(out=b_sb, in_=bqkv)
    nc.sync.dma_start(out=tri_sb, in_=tri2)
    for b, tg in [(0, 1), (1, 0), (1, 1)]:
        t0 = tg * 1024
        for kc in range(8):
            nc.sync.dma_start(out=xps[b][:, kc, t0:t0 + 1024],
                              in_=xsrcs[b][:, kc, t0:t0 + 1024])
    nc.sync.dma_start(out=wp_sb, in_=wp)

    filler = deque()

    def pop_filler(n=1):
        for _ in range(n):
            if filler:
                filler.popleft()[1]()

    def drain_until(key):
        """Pop everything up to and including the last item tagged `key`."""
        if not any(k == key for k, _ in filler):
            return
        while filler:
            k, th = filler.popleft()
            th()
            if k == key and not any(k2 == key for k2, _ in filler):
                break

    qkv_t = {}

    def make_qkv(b):
        """qkvT tiles + thunks per (tg, m, n): one 8-MM dense chain each."""
        dsts = [qkvpool.tile([128, T], bf16, tag="qkv", name=f"qkv{b}_{m}")
                for m in range(3)]
        qkv_t[b] = dsts
        thunks = {0: [], 1: []}
        for tg in range(2):
            for m in (2, 1, 0):    # v first: unblocks the vt transpose
                for n in range(2):
                    def th(m=m, tg=tg, n=n):
                        t0 = tg * 1024 + n * QS
                        pg = miscp.tile([128, QS], f32, tag="mm", name="pg")
                        for kc in range(8):
                            nc.tensor.matmul(
                                pg[:, :],
                                w_sb[:, kc, 128 * m:128 * m + 128],
                                xps[b][:, kc, t0:t0 + QS],
                                start=(kc == 0), stop=(kc == 7),
                            )
                        nc.vector.tensor_scalar_add(
                            dsts[m][:, t0:t0 + QS], pg[:, :],
                            b_sb[:, m:m + 1])
                    thunks[tg].append(th)
        return thunks

    vt_t = {}

    def make_vt(b):
        vt = vtmpool.tile([128, 16, HPC, 65], bf16, tag="vtm", name=f"vt{b}")
        vt_t[b] = vt
        nc.vector.memset(vt[:, :, :, 64:65], 1.0)

        def make_th(g, h):
            def th():
                # XBAR transpose needs a contiguous output tile; DVE then
                # scatters into the interleaved (v|1) layout.
                sc = vscr.tile([128, 8, 64], bf16, tag="vscr")
                nc.sync.dma_start_transpose(
                    sc[:, :, :],
                    qkv_t[b][2][64 * h:64 * h + 64, 1024 * g:1024 * g + 1024])
                nc.vector.tensor_copy(
                    out=vt[:, 8 * g:8 * g + 8, h, 0:64], in_=sc[:, :, :])
            return th
        return {g: [make_th(g, h) for h in range(HPC)] for g in range(2)}

    def make_proj(b, s, un):
        """Projection thunks, one per 128-token strip."""
        thunks = []
        for ts in range(QS // 128):
            def th(ts=ts):
                a0 = s * QS + ts * 128
                for ct in range(2):
                    pp = miscp.tile([128, 512], f32, tag="mm", name="pp")
                    nc.tensor.matmul(
                        pp[:, :],
                        un[:, ts * 128:(ts + 1) * 128],
                        wp_sb[:, ct * 512:(ct + 1) * 512],
                        start=True, stop=True,
                    )
                    ob = outsb.tile([128, 512], bf16, tag="osb")
                    if (ts * 2 + ct) % 4 == 3:  # 1/4 on ScalarE: balance DVE
                        nc.scalar.copy(ob[:, :], pp[:, :])
                    else:
                        nc.vector.tensor_copy(ob[:, :], pp[:, :])
                    nc.sync.dma_start(
                        out=outp[b, a0:a0 + 128, ct * 512:(ct + 1) * 512],
                        in_=ob[:, :])
            thunks.append(th)
        return thunks

    def emit_attention(b, s):
        qT_t, kT_t, vT_t = qkv_t[b]
        vt = vt_t[b]
        q0 = s * QS
        nkc = 4 * (s + 1)
        ot = [otp.tile([65, QS], f32, tag=f"ot{h}", name=f"ot{h}")
              for h in range(HPC)]

        def emit_o(kc):
            # One MM per (chunk, head) covering [ls:QS): the tri-masked pt
            # zeroes invalid diagonal-strip contributions, so no region
            # split is needed. kc=0 always covers the full [0:QS) width.
            ls = max(0, kc * KC - q0)
            last = kc == nkc - 1
            pt = pts[kc]
            for h in range(HPC):
                nc.tensor.matmul(
                    ot[h][:, ls:QS], vt[:, kc, h, :], pt[:, h, ls:QS],
                    start=(kc == 0), stop=last)

        pts = {}
        for kc in range(nkc):
            k0 = kc * KC
            ls = max(0, k0 - q0)
            st = stp.tile([128, HPC, QS], f32, tag="st")
            for h in range(HPC):
                nc.tensor.matmul(
                    st[:, h, ls:QS],
                    kT_t[64 * h:64 * h + 64, k0:k0 + KC],
                    qT_t[64 * h:64 * h + 64, q0 + ls:q0 + QS],
                    start=True, stop=True, tile_position=(64 * h, 0))
            pt = ptpool.tile([128, HPC, QS], bf16, tag="pt")
            pts[kc] = pt
            nc.scalar.activation(
                pt[:, :, ls:QS], st[:, :, ls:QS], AF.Exp, scale=SCALE)
            if kc >= 4 * s:  # diagonal chunk: zero invalid triangle, 2 heads
                nc.vector.tensor_mul(
                    pt[:, :, ls:ls + 128], pt[:, :, ls:ls + 128],
                    tri_sb[:, :])
            # O lags one chunk so exp(kc) overlaps PE work
            if kc > 0:
                emit_o(kc - 1)
                pts.pop(kc - 1)
            pop_filler()
        emit_o(nkc - 1)
        pop_filler()

        # normalization: un[64h:64h+64] = ot_h[0:64] / ot_h[64]
        un = unormp.tile([128, QS], bf16, tag="un", name=f"un{b}{s}")
        se = rows.tile([1, 2 * QS], f32, tag="se", name="se")
        for h in range(HPC):
            nc.vector.tensor_copy(se[:, h * QS:(h + 1) * QS], ot[h][64:65, :])
        rc = rows.tile([1, 2 * QS], f32, tag="rc", name="rc")
        nc.vector.reciprocal_approx_fast(rc[:, :], se[:, :])
        for h in range(HPC):
            rb = rows.tile([64, QS], f32, tag=f"rb{h}", name=f"rb{h}")
            nc.gpsimd.partition_broadcast(rb[:, :], rc[:, h * QS:(h + 1) * QS])
            nc.vector.tensor_mul(
                un[64 * h:64 * h + 64, :], ot[h][0:64, :], rb[:, :])
        return un

    # front: batch 0, first half QKV + vt, emitted densely
    th0 = make_qkv(0)
    vth0 = make_vt(0)
    for th in th0[0]:
        th()
    for th in vth0[0]:
        th()

    # filler: remaining QKV/vt work, phased one step ahead of consumption
    th1 = make_qkv(1)
    vth1 = make_vt(1)
    filler.extend((("qkv", 1, 0), th) for th in th1[0])
    filler.extend((("vt", 1, 0), th) for th in vth1[0])
    filler.extend((("qkv", 0, 1), th) for th in th0[1])
    filler.extend((("vt", 0, 1), th) for th in vth0[1])
    filler.extend((("qkv", 1, 1), th) for th in th1[1])
    filler.extend((("vt", 1, 1), th) for th in vth1[1])

    held = []
    for b, s in [(0, 0), (0, 1), (1, 0), (1, 1),
                 (0, 2), (0, 3), (1, 2), (1, 3)]:
        g = 0 if s < 2 else 1
        drain_until(("qkv", b, g))
        drain_until(("vt", b, g))
        un = emit_attention(b, s)
        pthunks = make_proj(b, s, un)
        if (b, s) == (1, 3):
            held.extend(pthunks)
        else:
            filler.extend((("proj", b, s), th) for th in pthunks)

    while filler:
        pop_filler()
    for th in held:
        th()


def build():
    if "nc" in _CACHE:
        return _CACHE["nc"]
    nc = bacc.Bacc("TRN2", target_bir_lowering=False, debug=False,
                   num_devices=NCORES)
    with tile.TileContext(nc) as tc:
        _emit(tc)
    nc.compile()
    _CACHE["nc"] = nc
    return nc


def make_in_maps(x, qkv_w, qkv_b, proj_w):
    import ml_dtypes
    bf16 = ml_dtypes.bfloat16
    x = np.asarray(x, dtype=np.float32)
    qkv_w = np.asarray(qkv_w, dtype=np.float32)
    qkv_b = np.asarray(qkv_b, dtype=np.float32)
    proj_w = np.asarray(proj_w, dtype=np.float32)

    xT = np.ascontiguousarray(x.transpose(0, 2, 1)).astype(bf16)
    tri = (np.arange(128)[None, :] >= np.arange(128)[:, None]).astype(bf16)
    tri2 = np.ascontiguousarray(np.tile(tri, (1, 2)))

    in_maps = []
    for c in range(NCORES):
        s = 64 * HPC * c  # first feature row of this core's heads
        wq = qkv_w[:, s:s + 128]
        wk = qkv_w[:, C + s:C + s + 128]
        wv = qkv_w[:, 2 * C + s:2 * C + s + 128]
        wqkv_c = np.ascontiguousarray(
            np.concatenate([wq, wk, wv], axis=1)).astype(bf16)
        bqkv_c = np.ascontiguousarray(np.stack(
            [qkv_b[s:s + 128], qkv_b[C + s:C + s + 128],
             qkv_b[2 * C + s:2 * C + s + 128]], axis=1))
        wp_c = np.ascontiguousarray(proj_w[s:s + 128, :]).astype(bf16)
        in_maps.append({
            "xT": xT, "wqkv": wqkv_c, "bqkv": bqkv_c, "wp": wp_c,
            "tri2": tri2,
        })
    return in_maps


def kernel(x, qkv_w, qkv_b, proj_w, proj_b, _trace=False):
    nc = build()
    in_maps = make_in_maps(x, qkv_w, qkv_b, proj_w)
    res = run_bass_kernel_spmd(nc, in_maps, core_ids=list(range(NCORES)),
                               trace=_trace)
    acc = np.zeros((B, T, C), dtype=np.float64)
    for c in range(NCORES):
        acc += res.results[c]["outp"].astype(np.float64)
    acc += np.asarray(proj_b, dtype=np.float64)
    out = acc.astype(np.float32)
    _CACHE["last_results"] = res
    return out
